# revision 73
# baseline (speedup 1.0000x reference)
"""Deformable Conv2d (3x3, pad=1, stride=1) on Trainium2 — Bass/Tile kernel.

Sharding: data-parallel over batch across 8 NeuronCores (B=8 -> 1 image/core);
weights replicated. Per-core pipeline (all 16-bit work in fp16):
  host prep: xq [4488, 1024] fp16 difference planes [X | Dy | Dx | Dxy] on the
             zero-padded 68x66 grid (y in [-2,65], x in [-1,64]).  One 2KB
             gather descriptor at idx = 66*y0 + x0 + 133 fetches everything a
             bilinear sample needs:
                 samp = X + fy*Dy + fx*(Dx + fy*Dxy)
             and the grid's zero borders implement the reference's zero-pad
             semantics with NO corner masking (y0/x0 just clamp to [-2, 64]).
  per-chunk prep (pipelined one chunk ahead of the main loop; chunk sizes
  4,4,8,8,8 j-groups — small leading chunks cut the startup latency):
    phase A: offset conv (18ch 3x3) as PSUM-accumulated PE matmuls with
             contiguous rhs windows over the padded-66 grid.
    phase B: sampling coords -> fracs FF/FXY + gather indices on DVE in a
             pixel-major layout (partition = pixel%128); floor() via the
             1.5*2^23 magic-add trick (the -0.5 must stay a separate op:
             MAGIC-0.5 is not representable in fp32); indices wrapped
             p -> (p%16, p//16) and replicated to all 8 16-partition groups
             (each SWDGE queue reads its own group on real hardware).
  per-chunk main loop, gathers emitted one tap ahead of consumes; nothing
  data-dependent ever runs on Pool, so SWDGE descriptor generation is never
  blocked (and note TensorScalarPtr is NOT a legal Pool opcode on HW):
    per tap: one SWDGE dma_gather (2KB descriptors, pixel-major); per
    128-pixel group 3 scale ops m = f*D (DVE tensor_scalar in the 4x perf
    mode, 1/4 on ACT as activation scales; fracs are per-partition scalars)
    + 3 quad-wide DVE adds in a 2-deep tree; PE transposes to channel-major
    (pairs share a PSUM tile, ACT copies back); main conv as PSUM-accumulated
    fp16 matmuls (contraction = (channel, tap), 36 accumulation steps).
"""
import sys

sys.path.insert(0, "/opt/trn_rl_repo")

import numpy as np

import concourse.mybir as mybir
from concourse import bacc
from concourse import bass_utils
from concourse.tile import TileContext
from concourse.bass_types import AP
from concourse.masks import make_identity

B, C, O, H, W = 8, 256, 256, 64, 64
HW = H * W                  # 4096
NCORES = 8
NCHUNK = 4                  # pixel chunks in the main loop
CH = HW // NCHUNK           # 1024 pixels / chunk
JG = CH // 128              # 8 j-groups of 128 pixels / chunk
CR = H // NCHUNK            # 16 image rows / chunk
W2 = W + 2                  # padded row width (66)
GY, GX = H + 4, W + 2       # plane grid 68 x 66
XROWS = GY * GX             # 4488 plane rows
MAGIC = 12582912.0          # 1.5 * 2^23: float32 round-to-int bias
LEAD = 1                    # taps the gather stream leads the consume stream
AluOp = mybir.AluOpType


def _emit(nc):
    f32, f16, i16 = mybir.dt.float32, mybir.dt.float16, mybir.dt.int16

    x16 = nc.dram_tensor("x16", [2, 128, H * W], f16, kind="ExternalInput")
    xq = nc.dram_tensor("xq", [XROWS, 1024], f16, kind="ExternalInput")
    offw = nc.dram_tensor("offw", [128, 2, 9, 18], f16, kind="ExternalInput")
    offb = nc.dram_tensor("offb", [18, 1], f32, kind="ExternalInput")
    convw = nc.dram_tensor("convw", [128, 18, 256], f16, kind="ExternalInput")
    kgrid_d = nc.dram_tensor("kgrid", [128, 32, 18], f32, kind="ExternalInput")
    y_out = nc.dram_tensor("y", [O, H * W], f16, kind="ExternalOutput")

    with TileContext(nc) as tc:
        with tc.tile_pool(name="consts", bufs=1) as consts, \
             tc.tile_pool(name="pb", bufs=1) as pb, \
             tc.tile_pool(name="gather", bufs=3) as gp, \
             tc.tile_pool(name="outp", bufs=2) as op_pool, \
             tc.tile_pool(name="ps_a", bufs=1, space="PSUM") as ps_a, \
             tc.tile_pool(name="ps_tp", bufs=3, space="PSUM") as ps_tp, \
             tc.tile_pool(name="ps_acc", bufs=1, space="PSUM") as ps_acc:
            # PSUM banks: accs 4 + stp 2 + pa/offt (shared tag) 2 = 8

            # ---- small consts first (offw gates phase A's first matmul,
            # and these cost <1.5us of queue ahead of the x16 load) ----
            ident = consts.tile([128, 128], f16)
            make_identity(nc, ident)
            ident_f32 = consts.tile([128, 128], f32)
            make_identity(nc, ident_f32)
            offw_sb = consts.tile([128, 2, 9, 18], f16)
            nc.sync.dma_start(out=offw_sb[:], in_=offw.ap())
            offb_sb = consts.tile([18, 1], f32)
            nc.sync.dma_start(out=offb_sb[:], in_=offb.ap())
            kgrid = consts.tile([128, 32, 18], f32)
            nc.sync.dma_start(out=kgrid[:], in_=kgrid_d.ap())

            # ---- padded fp16 image: it gates phase A of chunk 0 ----
            x_pad = pb.tile([128, 2, H + 3, W2], f16)
            nc.gpsimd.memset(x_pad[:, :, 0, :], 0.0)
            nc.gpsimd.memset(x_pad[:, :, H + 1:H + 3, :], 0.0)
            nc.gpsimd.memset(x_pad[:, :, 1:H + 1, 0], 0.0)
            nc.gpsimd.memset(x_pad[:, :, 1:H + 1, W + 1], 0.0)
            for cc in range(2):
                for rh in range(2):
                    r0 = rh * (H // 2)
                    nc.sync.dma_start(
                        out=x_pad[:, cc, 1 + r0:1 + r0 + H // 2, 1:W + 1],
                        in_=x16.ap()[cc].rearrange(
                            "c (h w) -> c h w", h=H)[:, r0:r0 + H // 2, :])
            x_flat = x_pad.rearrange("c cc h w -> c cc (h w)")

            # ---- PE p-state warm-up: ~3us of dummy transposes so the
            # offset conv of chunk 0 runs at full clock from the start ----
            for _ in range(6):
                warm = ps_tp.tile([128, 128], f16, tag="stp", name="warm")
                nc.tensor.transpose(warm[:], ident[:], ident[:])

            # ================= per-chunk prep =================
            # chunks are (first j-group, j-group count); the small first
            # chunk cuts startup latency, the small last one cuts the drain
            CHUNKS = [(0, 4), (4, 4), (8, 8), (16, 8), (24, 4), (28, 4)]

            def phase_a(c, jl0, njl):
                """Offset conv for image rows [2*jl0, 2*(jl0+njl))."""
                rows_all = njl * 2
                off66 = pb.tile([18, rows_all * W2], f32, name=f"off66_{c}",
                                tag=f"off66_{c}")
                tiles = (((0, 4), (4, 4)) if rows_all == 8
                         else ((0, 7), (7, 7), (14, 2)))
                for r0, rows in tiles:
                    n = rows * W2
                    pa = ps_a.tile([18, 462], f32, tag="pa", name="pa")
                    # cc outer: the first 9 matmuls only need the first half
                    # of x16, so phase A starts before the second DMA lands
                    for cc in range(2):
                        for k in range(9):
                            ky, kx = k // 3, k % 3
                            base = (jl0 * 2 + r0 + ky) * W2 + kx
                            nc.tensor.matmul(
                                pa[:, 0:n],
                                offw_sb[:, cc, k, :],
                                x_flat[:, cc, base:base + n],
                                start=(k == 0 and cc == 0),
                                stop=(k == 8 and cc == 1))
                    nc.vector.tensor_scalar(
                        out=off66[:, r0 * W2:r0 * W2 + n], in0=pa[:, 0:n],
                        scalar1=offb_sb[:, 0:1], scalar2=None, op0=AluOp.add)
                return off66

            def phase_b(c, off66, jl0, njl):
                """Sampling fracs FF + plane gather indices for chunk c."""
                # pixel-major offsets: offpx[q, jl, ch] (p = (jl0+jl)*128+q)
                offpx = pb.tile([128, njl, 18], f32, name=f"offpx_{c}",
                                tag=f"offpx_{c}")
                for hl in range(njl * 2):
                    pt = ps_tp.tile([64, 18], f32, tag="stp", name="offt")
                    nc.tensor.transpose(
                        pt[:], off66[:, hl * W2:hl * W2 + W],
                        ident_f32[0:18, 0:18])
                    nc.scalar.copy(
                        offpx[(hl % 2) * 64:(hl % 2) * 64 + 64, hl // 2, :],
                        pt[:])

                shp = [128, njl, 18]
                PP = pb.tile(shp, f32, name=f"PP_{c}", tag="PP")
                II = pb.tile(shp, f32, name=f"II_{c}", tag="II")
                T1 = pb.tile(shp, f32, name=f"T1_{c}", tag="T1")
                FF = pb.tile(shp, f32, name=f"FF_{c}", tag=f"FF_{c}")
                FXY = pb.tile([128, njl, 9], f32, name=f"FXY_{c}",
                              tag=f"FXY_{c}")
                tb = pb.tile([128, njl, 9], f32, name=f"tb_{c}", tag="tb")
                idx16 = pb.tile([128, 9, njl], i16, name=f"idx16_{c}",
                                tag="idx16")

                def ts(out, in0, s, op):
                    nc.vector.tensor_scalar(out=out, in0=in0, scalar1=s,
                                            scalar2=None, op0=op)

                nc.vector.tensor_add(PP[:], offpx[:],
                                     kgrid[:, jl0:jl0 + njl, :])
                ts(T1[:], PP[:], 0.5, AluOp.subtract)
                ts(T1[:], T1[:], MAGIC, AluOp.add)
                ts(II[:], T1[:], MAGIC, AluOp.subtract)    # II = floor(PP)
                # index path first (it gates the wrap DMAs and descgen);
                # clamp into T1 so II stays unclamped for the fracs below.
                # The grid's zero borders make OOB samples contribute zero.
                ts(T1[:], II[:], -2.0, AluOp.max)
                ts(T1[:], T1[:], 64.0, AluOp.min)
                nc.vector.tensor_scalar(
                    out=tb[:], in0=T1[:, :, 0:18:2],
                    scalar1=float(GX), scalar2=133.0,
                    op0=AluOp.mult, op1=AluOp.add)
                nc.vector.tensor_add(tb[:], tb[:], T1[:, :, 1:18:2])
                ts(tb[:], tb[:], 0.0, AluOp.max)
                nc.vector.tensor_copy(
                    idx16[:].rearrange("p k j -> p j k"), tb[:])
                nc.vector.tensor_sub(FF[:], PP[:], II[:])  # frac in [0,1)
                nc.vector.tensor_mul(FXY[:], FF[:, :, 0:18:2],
                                     FF[:, :, 1:18:2])     # fy*fx per tap

                # wrapped gather indices: position p -> (p%16, p//16),
                # replicated to all 8 16-partition groups (each SWDGE queue
                # reads its own group on real hardware).
                idxw = pb.tile([128, 9, njl * 8], i16, name=f"idxw_{c}",
                               tag=f"idxw_{c}")
                for qh in range(8):
                    nc.sync.dma_start(
                        out=idxw[0:16, :, qh:njl * 8:8],
                        in_=idx16[qh * 16:(qh + 1) * 16, :, :])
                for m in range(1, 8):
                    nc.sync.dma_start(out=idxw[m * 16:(m + 1) * 16, :, :],
                                      in_=idxw[0:16, :, :])
                return FF, FXY, idxw

            def prep(c):
                jl0, njl = CHUNKS[c]
                return phase_b(c, phase_a(c, jl0, njl), jl0, njl)

            # chunk-0 prep first (highest priority: first gathers gate all)
            preps = {0: prep(0)}

            # main-conv weights are not needed until the first matmul of the
            # main loop — load them after chunk-0 prep so they stay off the
            # startup critical path.
            convw_sb = consts.tile([128, 18, 256], f16)
            nc.sync.dma_start(out=convw_sb[:], in_=convw.ap())

            # ================= main loop =================
            xt_win = AP(tensor=xq, offset=0, ap=[[1024, XROWS], [1, 1024]])
            for ch, (jl0, njl) in enumerate(CHUNKS):
                # prep for the next chunk goes first: it fills engine idle
                # slots while this chunk's first gathers are in flight
                if ch + 1 < len(CHUNKS):
                    preps[ch + 1] = prep(ch + 1)
                FF, FXY, idxw = preps[ch]
                chp = njl * 128
                nsub = njl // 4
                accs = [ps_acc.tile([128, 512], f32, tag=f"acc{a}",
                                    name=f"acc{a}")
                        for a in range(2 * nsub)]
                gs = {}

                def gather(k):
                    # k==0 gets its own tag: its buffer WARs only against the
                    # previous chunk's k==0 gather (long freed), so a chunk's
                    # first gather never waits on the consume backlog
                    if k in (0, 8):
                        g = gp.tile([128, njl, 1024], f16, tag="g0",
                                    name="g0", bufs=2)
                    else:
                        g = gp.tile([128, njl, 1024], f16, tag="g", name="g",
                                    bufs=LEAD + 3)
                    nc.gpsimd.dma_gather(
                        out_ap=g[:], in_ap=xt_win,
                        idxs_ap=idxw[:, k, :],
                        num_idxs=chp, num_idxs_reg=chp,
                        elem_size=1024, elem_step=1024,
                        transpose=False)
                    return g

                def consume(k):
                    g = gs.pop(k)
                    s_t = gp.tile([128, njl, 256], f16, tag="s", name="s",
                                  bufs=4)
                    sk = gp.tile([128, 2, chp], f16, tag="sk", name="sk",
                                 bufs=3)
                    # per unit: 3 scale ops m = f*D (DVE tensor_scalar runs
                    # at the 4x perf mode; 1/4 go to ACT as activation
                    # scales) + 3 quad-wide DVE adds (4 j-groups per op).
                    for q in range(nsub):
                        j0 = q * 4
                        ms = [gp.tile([128, 4, 256], f16, tag=f"m{t}",
                                      name=f"m{t}", bufs=2) for t in range(3)]
                        for i in range(4):
                            j = j0 + i
                            srcs = (
                                (g[:, j, 256:512], FF[:, j, 2 * k:2 * k + 1]),
                                (g[:, j, 512:768],
                                 FF[:, j, 2 * k + 1:2 * k + 2]),
                                (g[:, j, 768:1024], FXY[:, j, k:k + 1]),
                            )
                            tail = (ch == len(CHUNKS) - 1 and k >= 6)
                            for t, (src, sc) in enumerate(srcs):
                                if not tail and (i + t + k) % 4 == 3:
                                    nc.scalar.activation(
                                        ms[t][:, i, :], src,
                                        mybir.ActivationFunctionType.Copy,
                                        scale=sc)
                                else:
                                    nc.vector.tensor_scalar(
                                        out=ms[t][:, i, :], in0=src,
                                        scalar1=sc, scalar2=None,
                                        op0=AluOp.mult)
                        # two-deep add tree: (X+m0) + (m1+m2)
                        sq = s_t[:, j0:j0 + 4, :]
                        nc.vector.tensor_add(sq, g[:, j0:j0 + 4, 0:256],
                                             ms[0][:])
                        nc.vector.tensor_add(ms[1][:], ms[1][:], ms[2][:])
                        nc.vector.tensor_add(sq, sq, ms[1][:])
                        for i in range(4):
                            j = j0 + i
                            ptp = ps_tp.tile([128, 2, 128], f16, tag="stp",
                                             name="stp")
                            for cc in range(2):
                                nc.tensor.transpose(
                                    ptp[:, cc, :],
                                    s_t[:, j, cc * 128:(cc + 1) * 128],
                                    ident[:])
                            nc.scalar.copy(
                                sk[:, :, j * 128:(j + 1) * 128], ptp[:])
                    for cc in range(2):
                        for o in range(2):
                            for sub in range(nsub):
                                nc.tensor.matmul(
                                    accs[o * nsub + sub],
                                    convw_sb[:, k * 2 + cc,
                                             o * 128:(o + 1) * 128],
                                    sk[:, cc, sub * 512:(sub + 1) * 512],
                                    start=(k == 0 and cc == 0),
                                    stop=(k == 8 and cc == 1))

                # software pipeline: gathers lead consumes by LEAD taps so
                # SWDGE descgen on Pool is never queued behind combine work
                for k in range(9):
                    gs[k] = gather(k)
                    if k >= LEAD:
                        consume(k - LEAD)
                for k in range(9 - LEAD, 9):
                    consume(k)

                for o in range(2):
                    ob = op_pool.tile([128, chp], f16, tag=f"ob{o}",
                                      name=f"ob{o}")
                    for sub in range(nsub):
                        if o == 0:
                            nc.scalar.copy(ob[:, sub * 512:(sub + 1) * 512],
                                           accs[o * nsub + sub][:])
                        else:
                            nc.vector.tensor_copy(
                                ob[:, sub * 512:(sub + 1) * 512],
                                accs[o * nsub + sub][:])
                    nc.sync.dma_start(
                        out=y_out.ap()[o * 128:(o + 1) * 128,
                                       jl0 * 128:jl0 * 128 + chp],
                        in_=ob[:])
    nc.compile()
    return nc


_CACHE = {}


def _get_nc():
    if "nc" not in _CACHE:
        nc = bacc.Bacc("TRN2", target_bir_lowering=False, debug=False,
                       num_devices=NCORES)
        _CACHE["nc"] = _emit(nc)
    return _CACHE["nc"]


def _host_tables():
    if "kgrid" in _CACHE:
        return _CACHE["kgrid"]
    q = np.arange(128)[:, None, None]
    j = np.arange(32)[None, :, None]
    c = np.arange(18)[None, None, :]
    p = j * 128 + q
    k = c // 2
    d = c % 2
    ky, kx = k // 3, k % 3
    grid = np.where(d == 0, p // W + ky - 1, p % W + kx - 1).astype(np.float32)
    _CACHE["kgrid"] = np.ascontiguousarray(grid)
    return _CACHE["kgrid"]


def _pack_weights(offset_w, offset_b, conv_w):
    # offw lhsT: [c, cc, k, j] = offset_w[j, cc*128+c, ky, kx]
    ow = offset_w.reshape(18, 2, 128, 9).transpose(2, 1, 3, 0)
    # convw lhsT: [c, (k,cc) chunk, o] = conv_w[o, cc*128+c, k]
    cw = conv_w.reshape(256, 2, 128, 9).transpose(2, 3, 1, 0)  # c, k, cc, o
    cw = cw.reshape(128, 18, 256)
    ob = offset_b.reshape(18, 1)
    return (np.ascontiguousarray(ow, np.float16),
            np.ascontiguousarray(ob, np.float32),
            np.ascontiguousarray(cw, np.float16))


def _pack_planes(xb16):
    """xb16: [256, HW] fp16 -> xq [4488, 1024] difference planes.

    Grid row R = gy*66 + gx covers pixel (y, x) = (gy-2, gx-1); planes are
    [X | Dy | Dx | Dxy] of the zero-padded fp16 image, so
    X[R] + fy*Dy[R] + fx*(Dx[R] + fy*Dxy[R]) is the exact zero-padded
    bilinear sample at (y+fy, x+fx).
    """
    Xp = np.zeros((GY + 1, GX + 1, C), np.float32)
    Xp[2:2 + H, 1:1 + W] = xb16.reshape(C, H, W).transpose(1, 2, 0)
    Xs = Xp[0:GY, 0:GX]
    Dy = Xp[1:GY + 1, 0:GX] - Xs
    Dx = Xp[0:GY, 1:GX + 1] - Xs
    Dxy = Xp[1:GY + 1, 1:GX + 1] - Xp[0:GY, 1:GX + 1] - Xp[1:GY + 1, 0:GX] + Xs
    xq = np.concatenate(
        [A.reshape(XROWS, C) for A in (Xs, Dy, Dx, Dxy)],
        axis=1).astype(np.float16)
    return xq


def make_in_maps(x, offset_w, offset_b, conv_w):
    ow, ob, cw = _pack_weights(np.asarray(offset_w), np.asarray(offset_b),
                               np.asarray(conv_w))
    kg = _host_tables()
    x16 = np.asarray(x, np.float32).reshape(B, 2, 128, HW).astype(np.float16)
    maps = []
    for b in range(B):
        maps.append({
            "x16": np.ascontiguousarray(x16[b]),
            "xq": _pack_planes(x16[b].reshape(256, HW)),
            "offw": ow, "offb": ob, "convw": cw, "kgrid": kg,
        })
    return maps


def postprocess(res):
    out = np.stack([np.asarray(res.results[b]["y"]).astype(np.float32)
                    .reshape(O, H, W) for b in range(B)])
    return out


def kernel(x, offset_w, offset_b, conv_w):
    nc = _get_nc()
    in_maps = make_in_maps(x, offset_w, offset_b, conv_w)
    res = bass_utils.run_bass_kernel_spmd(nc, in_maps,
                                          core_ids=list(range(NCORES)))
    return postprocess(res)


# revision 82
# speedup vs baseline: 1.0009x; 1.0009x over previous
"""Deformable Conv2d (3x3, pad=1, stride=1) on Trainium2 — Bass/Tile kernel.

Sharding: data-parallel over batch across 8 NeuronCores (B=8 -> 1 image/core);
weights replicated. Per-core pipeline (all 16-bit work in fp16):
  host prep: xq [4488, 1024] fp16 difference planes [X | Dy | Dx | Dxy] on the
             zero-padded 68x66 grid (y in [-2,65], x in [-1,64]).  One 2KB
             gather descriptor at idx = 66*y0 + x0 + 133 fetches everything a
             bilinear sample needs:
                 samp = X + fy*Dy + fx*(Dx + fy*Dxy)
             and the grid's zero borders implement the reference's zero-pad
             semantics with NO corner masking (y0/x0 just clamp to [-2, 64]).
  per-chunk prep (pipelined one chunk ahead of the main loop; chunk sizes
  4,4,8,8,8 j-groups — small leading chunks cut the startup latency):
    phase A: offset conv (18ch 3x3) as PSUM-accumulated PE matmuls with
             contiguous rhs windows over the padded-66 grid.
    phase B: sampling coords -> fracs FF/FXY + gather indices on DVE in a
             pixel-major layout (partition = pixel%128); floor() via the
             1.5*2^23 magic-add trick (the -0.5 must stay a separate op:
             MAGIC-0.5 is not representable in fp32); indices wrapped
             p -> (p%16, p//16) and replicated to all 8 16-partition groups
             (each SWDGE queue reads its own group on real hardware).
  per-chunk main loop, gathers emitted one tap ahead of consumes; nothing
  data-dependent ever runs on Pool, so SWDGE descriptor generation is never
  blocked (and note TensorScalarPtr is NOT a legal Pool opcode on HW):
    per tap: one SWDGE dma_gather (2KB descriptors, pixel-major); per
    128-pixel group 3 scale ops m = f*D (DVE tensor_scalar in the 4x perf
    mode, 1/4 on ACT as activation scales; fracs are per-partition scalars)
    + 3 quad-wide DVE adds in a 2-deep tree; PE transposes to channel-major
    (pairs share a PSUM tile, ACT copies back); main conv as PSUM-accumulated
    fp16 matmuls (contraction = (channel, tap), 36 accumulation steps).
"""
import sys

sys.path.insert(0, "/opt/trn_rl_repo")

import numpy as np

import concourse.mybir as mybir
from concourse import bacc
from concourse import bass_utils
from concourse.tile import TileContext
from concourse.bass_types import AP
from concourse.masks import make_identity

B, C, O, H, W = 8, 256, 256, 64, 64
HW = H * W                  # 4096
NCORES = 8
NCHUNK = 4                  # pixel chunks in the main loop
CH = HW // NCHUNK           # 1024 pixels / chunk
JG = CH // 128              # 8 j-groups of 128 pixels / chunk
CR = H // NCHUNK            # 16 image rows / chunk
W2 = W + 2                  # padded row width (66)
GY, GX = H + 4, W + 2       # plane grid 68 x 66
XROWS = GY * GX             # 4488 plane rows
MAGIC = 12582912.0          # 1.5 * 2^23: float32 round-to-int bias
LEAD = 1                    # taps the gather stream leads the consume stream
AluOp = mybir.AluOpType


def _emit(nc):
    f32, f16, i16 = mybir.dt.float32, mybir.dt.float16, mybir.dt.int16

    x16 = nc.dram_tensor("x16", [2, 128, H * W], f16, kind="ExternalInput")
    xq = nc.dram_tensor("xq", [XROWS, 1024], f16, kind="ExternalInput")
    offw = nc.dram_tensor("offw", [128, 2, 9, 18], f16, kind="ExternalInput")
    offb = nc.dram_tensor("offb", [18, 1], f32, kind="ExternalInput")
    convw = nc.dram_tensor("convw", [128, 18, 256], f16, kind="ExternalInput")
    kgrid_d = nc.dram_tensor("kgrid", [128, 32, 18], f32, kind="ExternalInput")
    y_out = nc.dram_tensor("y", [O, H * W], f16, kind="ExternalOutput")

    with TileContext(nc) as tc:
        with tc.tile_pool(name="consts", bufs=1) as consts, \
             tc.tile_pool(name="pb", bufs=1) as pb, \
             tc.tile_pool(name="gather", bufs=3) as gp, \
             tc.tile_pool(name="outp", bufs=2) as op_pool, \
             tc.tile_pool(name="ps_a", bufs=1, space="PSUM") as ps_a, \
             tc.tile_pool(name="ps_tp", bufs=3, space="PSUM") as ps_tp, \
             tc.tile_pool(name="ps_acc", bufs=1, space="PSUM") as ps_acc:
            # PSUM banks: accs 4 + stp 2 + pa/offt (shared tag) 2 = 8

            # ---- small consts first (offw gates phase A's first matmul,
            # and these cost <1.5us of queue ahead of the x16 load) ----
            ident = consts.tile([128, 128], f16)
            make_identity(nc, ident)
            ident_f32 = consts.tile([128, 128], f32)
            make_identity(nc, ident_f32)
            offw_sb = consts.tile([128, 2, 9, 18], f16)
            nc.sync.dma_start(out=offw_sb[:], in_=offw.ap())
            offb_sb = consts.tile([18, 1], f32)
            nc.sync.dma_start(out=offb_sb[:], in_=offb.ap())
            kgrid = consts.tile([128, 32, 18], f32)
            nc.sync.dma_start(out=kgrid[:], in_=kgrid_d.ap())

            # ---- padded fp16 image: it gates phase A of chunk 0 ----
            x_pad = pb.tile([128, 2, H + 3, W2], f16)
            nc.gpsimd.memset(x_pad[:, :, 0, :], 0.0)
            nc.gpsimd.memset(x_pad[:, :, H + 1:H + 3, :], 0.0)
            nc.gpsimd.memset(x_pad[:, :, 1:H + 1, 0], 0.0)
            nc.gpsimd.memset(x_pad[:, :, 1:H + 1, W + 1], 0.0)
            for cc in range(2):
                for rh in range(2):
                    r0 = rh * (H // 2)
                    nc.sync.dma_start(
                        out=x_pad[:, cc, 1 + r0:1 + r0 + H // 2, 1:W + 1],
                        in_=x16.ap()[cc].rearrange(
                            "c (h w) -> c h w", h=H)[:, r0:r0 + H // 2, :])
            x_flat = x_pad.rearrange("c cc h w -> c cc (h w)")

            # ---- PE p-state warm-up: ~3us of dummy transposes so the
            # offset conv of chunk 0 runs at full clock from the start ----
            for _ in range(6):
                warm = ps_tp.tile([128, 128], f16, tag="stp", name="warm")
                nc.tensor.transpose(warm[:], ident[:], ident[:])

            # ================= per-chunk prep =================
            # chunks are (first j-group, j-group count); the small first
            # chunk cuts startup latency, the small last one cuts the drain
            CHUNKS = [(0, 4), (4, 4), (8, 8), (16, 8), (24, 4), (28, 4)]

            def phase_a(c, jl0, njl):
                """Offset conv for image rows [2*jl0, 2*(jl0+njl))."""
                rows_all = njl * 2
                off66 = pb.tile([18, rows_all * W2], f32, name=f"off66_{c}",
                                tag=f"off66_{c}")
                tiles = (((0, 4), (4, 4)) if rows_all == 8
                         else ((0, 7), (7, 7), (14, 2)))
                for r0, rows in tiles:
                    n = rows * W2
                    pa = ps_a.tile([18, 462], f32, tag="pa", name="pa")
                    # cc outer: the first 9 matmuls only need the first half
                    # of x16, so phase A starts before the second DMA lands
                    for cc in range(2):
                        for k in range(9):
                            ky, kx = k // 3, k % 3
                            base = (jl0 * 2 + r0 + ky) * W2 + kx
                            nc.tensor.matmul(
                                pa[:, 0:n],
                                offw_sb[:, cc, k, :],
                                x_flat[:, cc, base:base + n],
                                start=(k == 0 and cc == 0),
                                stop=(k == 8 and cc == 1))
                    nc.vector.tensor_scalar(
                        out=off66[:, r0 * W2:r0 * W2 + n], in0=pa[:, 0:n],
                        scalar1=offb_sb[:, 0:1], scalar2=None, op0=AluOp.add)
                return off66

            def phase_b(c, off66, jl0, njl):
                """Sampling fracs FF + plane gather indices for chunk c."""
                # pixel-major offsets: offpx[q, jl, ch] (p = (jl0+jl)*128+q)
                offpx = pb.tile([128, njl, 18], f32, name=f"offpx_{c}",
                                tag=f"offpx_{c}")
                for hl in range(njl * 2):
                    pt = ps_tp.tile([64, 18], f32, tag="stp", name="offt")
                    nc.tensor.transpose(
                        pt[:], off66[:, hl * W2:hl * W2 + W],
                        ident_f32[0:18, 0:18])
                    nc.scalar.copy(
                        offpx[(hl % 2) * 64:(hl % 2) * 64 + 64, hl // 2, :],
                        pt[:])

                shp = [128, njl, 18]
                PP = pb.tile(shp, f32, name=f"PP_{c}", tag="PP")
                II = pb.tile(shp, f32, name=f"II_{c}", tag="II")
                T1 = pb.tile(shp, f32, name=f"T1_{c}", tag="T1")
                FF = pb.tile(shp, f32, name=f"FF_{c}", tag=f"FF_{c}")
                FXY = pb.tile([128, njl, 9], f32, name=f"FXY_{c}",
                              tag=f"FXY_{c}")
                tb = pb.tile([128, njl, 9], f32, name=f"tb_{c}", tag="tb")
                idx16 = pb.tile([128, 9, njl], i16, name=f"idx16_{c}",
                                tag="idx16")

                def ts(out, in0, s, op):
                    nc.vector.tensor_scalar(out=out, in0=in0, scalar1=s,
                                            scalar2=None, op0=op)

                nc.vector.tensor_add(PP[:], offpx[:],
                                     kgrid[:, jl0:jl0 + njl, :])
                ts(T1[:], PP[:], 0.5, AluOp.subtract)
                ts(T1[:], T1[:], MAGIC, AluOp.add)
                ts(II[:], T1[:], MAGIC, AluOp.subtract)    # II = floor(PP)
                # index path first (it gates the wrap DMAs and descgen);
                # clamp into T1 so II stays unclamped for the fracs below.
                # The grid's zero borders make OOB samples contribute zero.
                ts(T1[:], II[:], -2.0, AluOp.max)
                ts(T1[:], T1[:], 64.0, AluOp.min)
                nc.vector.tensor_scalar(
                    out=tb[:], in0=T1[:, :, 0:18:2],
                    scalar1=float(GX), scalar2=133.0,
                    op0=AluOp.mult, op1=AluOp.add)
                nc.vector.tensor_add(tb[:], tb[:], T1[:, :, 1:18:2])
                ts(tb[:], tb[:], 0.0, AluOp.max)
                nc.vector.tensor_copy(
                    idx16[:].rearrange("p k j -> p j k"), tb[:])
                nc.vector.tensor_sub(FF[:], PP[:], II[:])  # frac in [0,1)
                nc.vector.tensor_mul(FXY[:], FF[:, :, 0:18:2],
                                     FF[:, :, 1:18:2])     # fy*fx per tap

                # wrapped gather indices: position p -> (p%16, p//16),
                # replicated to all 8 16-partition groups (each SWDGE queue
                # reads its own group on real hardware).
                idxw = pb.tile([128, 9, njl * 8], i16, name=f"idxw_{c}",
                               tag=f"idxw_{c}")
                for qh in range(8):
                    nc.sync.dma_start(
                        out=idxw[0:16, :, qh:njl * 8:8],
                        in_=idx16[qh * 16:(qh + 1) * 16, :, :])
                for m in range(1, 8):
                    nc.sync.dma_start(out=idxw[m * 16:(m + 1) * 16, :, :],
                                      in_=idxw[0:16, :, :])
                return FF, FXY, idxw

            def prep(c):
                jl0, njl = CHUNKS[c]
                return phase_b(c, phase_a(c, jl0, njl), jl0, njl)

            # chunk-0 prep first (highest priority: first gathers gate all)
            preps = {0: prep(0)}

            # main-conv weights are not needed until the first matmul of the
            # main loop — load them after chunk-0 prep so they stay off the
            # startup critical path.
            convw_sb = consts.tile([128, 18, 256], f16)
            nc.sync.dma_start(out=convw_sb[:], in_=convw.ap())

            # ================= main loop =================
            xt_win = AP(tensor=xq, offset=0, ap=[[1024, XROWS], [1, 1024]])
            for ch, (jl0, njl) in enumerate(CHUNKS):
                # prep for the next chunk goes first: it fills engine idle
                # slots while this chunk's first gathers are in flight
                if ch + 1 < len(CHUNKS):
                    preps[ch + 1] = prep(ch + 1)
                FF, FXY, idxw = preps[ch]
                chp = njl * 128
                nsub = njl // 4
                accs = [ps_acc.tile([128, 512], f32, tag=f"acc{a}",
                                    name=f"acc{a}")
                        for a in range(2 * nsub)]
                gs = {}

                def gather(k):
                    # k==0 gets its own tag: its buffer WARs only against the
                    # previous chunk's k==0 gather (long freed), so a chunk's
                    # first gather never waits on the consume backlog
                    if k in (0, 8):
                        g = gp.tile([128, njl, 1024], f16, tag="g0",
                                    name="g0", bufs=2)
                    else:
                        g = gp.tile([128, njl, 1024], f16, tag="g", name="g",
                                    bufs=LEAD + 3)
                    nc.gpsimd.dma_gather(
                        out_ap=g[:], in_ap=xt_win,
                        idxs_ap=idxw[:, k, :],
                        num_idxs=chp, num_idxs_reg=chp,
                        elem_size=1024, elem_step=1024,
                        transpose=False)
                    return g

                def consume(k):
                    g = gs.pop(k)
                    s_t = gp.tile([128, njl, 256], f16, tag="s", name="s",
                                  bufs=4)
                    sk = gp.tile([128, 2, chp], f16, tag="sk", name="sk",
                                 bufs=3)
                    # per unit: 3 scale ops m = f*D (DVE tensor_scalar runs
                    # at the 4x perf mode; 1/4 go to ACT as activation
                    # scales) + 3 quad-wide DVE adds (4 j-groups per op).
                    for q in range(nsub):
                        j0 = q * 4
                        ms = [gp.tile([128, 4, 256], f16, tag=f"m{t}",
                                      name=f"m{t}", bufs=2) for t in range(3)]
                        for i in range(4):
                            j = j0 + i
                            srcs = (
                                (g[:, j, 256:512], FF[:, j, 2 * k:2 * k + 1]),
                                (g[:, j, 512:768],
                                 FF[:, j, 2 * k + 1:2 * k + 2]),
                                (g[:, j, 768:1024], FXY[:, j, k:k + 1]),
                            )
                            tail = (ch == len(CHUNKS) - 1 and k >= 5)
                            for t, (src, sc) in enumerate(srcs):
                                if not tail and (i + t + k) % 4 == 3:
                                    nc.scalar.activation(
                                        ms[t][:, i, :], src,
                                        mybir.ActivationFunctionType.Copy,
                                        scale=sc)
                                else:
                                    nc.vector.tensor_scalar(
                                        out=ms[t][:, i, :], in0=src,
                                        scalar1=sc, scalar2=None,
                                        op0=AluOp.mult)
                        # two-deep add tree: (X+m0) + (m1+m2)
                        sq = s_t[:, j0:j0 + 4, :]
                        nc.vector.tensor_add(sq, g[:, j0:j0 + 4, 0:256],
                                             ms[0][:])
                        nc.vector.tensor_add(ms[1][:], ms[1][:], ms[2][:])
                        nc.vector.tensor_add(sq, sq, ms[1][:])
                        for i in range(4):
                            j = j0 + i
                            ptp = ps_tp.tile([128, 2, 128], f16, tag="stp",
                                             name="stp")
                            for cc in range(2):
                                nc.tensor.transpose(
                                    ptp[:, cc, :],
                                    s_t[:, j, cc * 128:(cc + 1) * 128],
                                    ident[:])
                            nc.scalar.copy(
                                sk[:, :, j * 128:(j + 1) * 128], ptp[:])
                    for cc in range(2):
                        for o in range(2):
                            for sub in range(nsub):
                                nc.tensor.matmul(
                                    accs[o * nsub + sub],
                                    convw_sb[:, k * 2 + cc,
                                             o * 128:(o + 1) * 128],
                                    sk[:, cc, sub * 512:(sub + 1) * 512],
                                    start=(k == 0 and cc == 0),
                                    stop=(k == 8 and cc == 1))

                # software pipeline: gathers lead consumes by LEAD taps so
                # SWDGE descgen on Pool is never queued behind combine work
                for k in range(9):
                    gs[k] = gather(k)
                    if k >= LEAD:
                        consume(k - LEAD)
                for k in range(9 - LEAD, 9):
                    consume(k)

                for o in range(2):
                    ob = op_pool.tile([128, chp], f16, tag=f"ob{o}",
                                      name=f"ob{o}")
                    for sub in range(nsub):
                        if o == 0:
                            nc.scalar.copy(ob[:, sub * 512:(sub + 1) * 512],
                                           accs[o * nsub + sub][:])
                        else:
                            nc.vector.tensor_copy(
                                ob[:, sub * 512:(sub + 1) * 512],
                                accs[o * nsub + sub][:])
                    nc.sync.dma_start(
                        out=y_out.ap()[o * 128:(o + 1) * 128,
                                       jl0 * 128:jl0 * 128 + chp],
                        in_=ob[:])
    nc.compile()
    return nc


_CACHE = {}


def _get_nc():
    if "nc" not in _CACHE:
        nc = bacc.Bacc("TRN2", target_bir_lowering=False, debug=False,
                       num_devices=NCORES)
        _CACHE["nc"] = _emit(nc)
    return _CACHE["nc"]


def _host_tables():
    if "kgrid" in _CACHE:
        return _CACHE["kgrid"]
    q = np.arange(128)[:, None, None]
    j = np.arange(32)[None, :, None]
    c = np.arange(18)[None, None, :]
    p = j * 128 + q
    k = c // 2
    d = c % 2
    ky, kx = k // 3, k % 3
    grid = np.where(d == 0, p // W + ky - 1, p % W + kx - 1).astype(np.float32)
    _CACHE["kgrid"] = np.ascontiguousarray(grid)
    return _CACHE["kgrid"]


def _pack_weights(offset_w, offset_b, conv_w):
    # offw lhsT: [c, cc, k, j] = offset_w[j, cc*128+c, ky, kx]
    ow = offset_w.reshape(18, 2, 128, 9).transpose(2, 1, 3, 0)
    # convw lhsT: [c, (k,cc) chunk, o] = conv_w[o, cc*128+c, k]
    cw = conv_w.reshape(256, 2, 128, 9).transpose(2, 3, 1, 0)  # c, k, cc, o
    cw = cw.reshape(128, 18, 256)
    ob = offset_b.reshape(18, 1)
    return (np.ascontiguousarray(ow, np.float16),
            np.ascontiguousarray(ob, np.float32),
            np.ascontiguousarray(cw, np.float16))


def _pack_planes(xb16):
    """xb16: [256, HW] fp16 -> xq [4488, 1024] difference planes.

    Grid row R = gy*66 + gx covers pixel (y, x) = (gy-2, gx-1); planes are
    [X | Dy | Dx | Dxy] of the zero-padded fp16 image, so
    X[R] + fy*Dy[R] + fx*(Dx[R] + fy*Dxy[R]) is the exact zero-padded
    bilinear sample at (y+fy, x+fx).
    """
    Xp = np.zeros((GY + 1, GX + 1, C), np.float32)
    Xp[2:2 + H, 1:1 + W] = xb16.reshape(C, H, W).transpose(1, 2, 0)
    Xs = Xp[0:GY, 0:GX]
    Dy = Xp[1:GY + 1, 0:GX] - Xs
    Dx = Xp[0:GY, 1:GX + 1] - Xs
    Dxy = Xp[1:GY + 1, 1:GX + 1] - Xp[0:GY, 1:GX + 1] - Xp[1:GY + 1, 0:GX] + Xs
    xq = np.concatenate(
        [A.reshape(XROWS, C) for A in (Xs, Dy, Dx, Dxy)],
        axis=1).astype(np.float16)
    return xq


def make_in_maps(x, offset_w, offset_b, conv_w):
    ow, ob, cw = _pack_weights(np.asarray(offset_w), np.asarray(offset_b),
                               np.asarray(conv_w))
    kg = _host_tables()
    x16 = np.asarray(x, np.float32).reshape(B, 2, 128, HW).astype(np.float16)
    maps = []
    for b in range(B):
        maps.append({
            "x16": np.ascontiguousarray(x16[b]),
            "xq": _pack_planes(x16[b].reshape(256, HW)),
            "offw": ow, "offb": ob, "convw": cw, "kgrid": kg,
        })
    return maps


def postprocess(res):
    out = np.stack([np.asarray(res.results[b]["y"]).astype(np.float32)
                    .reshape(O, H, W) for b in range(B)])
    return out


def kernel(x, offset_w, offset_b, conv_w):
    nc = _get_nc()
    in_maps = make_in_maps(x, offset_w, offset_b, conv_w)
    res = bass_utils.run_bass_kernel_spmd(nc, in_maps,
                                          core_ids=list(range(NCORES)))
    return postprocess(res)
